# revision 5
# baseline (speedup 1.0000x reference)
"""ArcTanDistortion kernel for Trainium2 (8 NeuronCores, SPMD).

y = (2/pi) * atan(GAIN * x) / log(GAIN), elementwise over x of shape
(8, 2, 4194304) float32. Batch dim (8) is sharded across the 8 cores;
each core streams its 32 MiB shard HBM->SBUF, applies the scalar-engine
Arctan activation (fused input scale = GAIN) and a DVE multiply by the
output constant, and streams back. Memory bound: ~64 MiB of HBM traffic
per core.
"""

import os
import numpy as np

GAIN = 67.0
OUT_SCALE = float((2.0 / np.pi) / np.log(GAIN))

B, C, N = 8, 2, 4194304          # full input shape
PER_CORE = C * N                 # 8388608 elements per core
P = 128                          # SBUF partitions
M = 8192                         # free-dim elements per tile (4 MiB tiles)
T = PER_CORE // (P * M)          # 8 tiles per core
assert T * P * M == PER_CORE

N_CORES = 8

LAST_EXEC_NS = None              # set when ARCTAN_TRACE=1


def _build_nc(reps: int = 1):
    import concourse.bacc as bacc
    import concourse.mybir as mybir
    import concourse.tile as tile

    # Bacc (not raw Bass): its finalize() runs generate_event_semaphores,
    # which splits multi-sem waits — TRN2 allows only one sync wait per
    # instruction and this kernel's DMA deps need two.
    nc = bacc.Bacc()
    x_in = nc.dram_tensor("x", [T, P, M], mybir.dt.float32, kind="ExternalInput")
    y_out = nc.dram_tensor("y", [T, P, M], mybir.dt.float32, kind="ExternalOutput")

    with tile.TileContext(nc) as tc:
        with tc.tile_pool(name="io", bufs=5) as pool:
            for _ in range(reps):
                for i in range(T):
                    t = pool.tile([P, M], mybir.dt.float32)
                    nc.sync.dma_start(out=t[:], in_=x_in[i])
                    nc.scalar.activation(
                        t[:], t[:], mybir.ActivationFunctionType.Arctan, scale=GAIN
                    )
                    nc.vector.tensor_scalar_mul(t[:], t[:], OUT_SCALE)
                    nc.sync.dma_start(out=y_out[i], in_=t[:])
    nc.finalize()
    return nc


def kernel(x: np.ndarray) -> np.ndarray:
    global LAST_EXEC_NS
    from concourse.bass_utils import run_bass_kernel_spmd

    x = np.asarray(x, dtype=np.float32)
    assert x.shape == (B, C, N), x.shape

    nc = _build_nc()
    in_maps = [
        {"x": np.ascontiguousarray(x[i]).reshape(T, P, M)} for i in range(N_CORES)
    ]
    trace = os.environ.get("ARCTAN_TRACE", "0") == "1"
    rr = run_bass_kernel_spmd(nc, in_maps, list(range(N_CORES)), trace=trace)
    LAST_EXEC_NS = rr.exec_time_ns

    out = np.empty((B, C, N), dtype=np.float32)
    for i in range(N_CORES):
        out[i] = rr.results[i]["y"].reshape(C, N)
    return out


# revision 8
# speedup vs baseline: 1.0109x; 1.0109x over previous
"""ArcTanDistortion kernel for Trainium2 (8 NeuronCores, SPMD).

y = (2/pi) * atan(GAIN * x) / log(GAIN), elementwise over x of shape
(8, 2, 4194304) float32. Batch dim (8) is sharded across the 8 cores;
each core streams its 32 MiB shard HBM->SBUF, applies the scalar-engine
Arctan activation (fused input scale = GAIN) and a DVE multiply by the
output constant, and streams back. Memory bound: ~64 MiB of HBM traffic
per core.
"""

import numpy as np

GAIN = 67.0
OUT_SCALE = float((2.0 / np.pi) / np.log(GAIN))

B, C, N = 8, 2, 4194304          # full input shape
PER_CORE = C * N                 # 8388608 elements per core
P = 128                          # SBUF partitions
M = 8192                         # free-dim elements per tile (4 MiB tiles)
T = PER_CORE // (P * M)          # 8 tiles per core
assert T * P * M == PER_CORE

N_CORES = 8


def _build_nc(reps: int = 1):
    import concourse.bacc as bacc
    import concourse.mybir as mybir
    import concourse.tile as tile

    # Bacc (not raw Bass): its finalize() runs generate_event_semaphores,
    # which splits multi-sem waits — TRN2 allows only one sync wait per
    # instruction and this kernel's DMA deps need two.
    nc = bacc.Bacc()
    x_in = nc.dram_tensor("x", [T, P, M], mybir.dt.float32, kind="ExternalInput")
    y_out = nc.dram_tensor("y", [T, P, M], mybir.dt.float32, kind="ExternalOutput")

    with tile.TileContext(nc) as tc:
        with tc.tile_pool(name="io", bufs=5) as pool:
            for _ in range(reps):
                for i in range(T):
                    t = pool.tile([P, M], mybir.dt.float32)
                    nc.sync.dma_start(out=t[:], in_=x_in[i])
                    nc.scalar.activation(
                        t[:], t[:], mybir.ActivationFunctionType.Arctan, scale=GAIN
                    )
                    nc.vector.tensor_scalar_mul(t[:], t[:], OUT_SCALE)
                    nc.sync.dma_start(out=y_out[i], in_=t[:])
    nc.finalize()
    return nc


def kernel(x: np.ndarray) -> np.ndarray:
    from concourse.bass_utils import run_bass_kernel_spmd

    x = np.asarray(x, dtype=np.float32)
    assert x.shape == (B, C, N), x.shape

    nc = _build_nc()
    in_maps = [
        {"x": np.ascontiguousarray(x[i]).reshape(T, P, M)} for i in range(N_CORES)
    ]
    rr = run_bass_kernel_spmd(nc, in_maps, list(range(N_CORES)))

    out = np.empty((B, C, N), dtype=np.float32)
    for i in range(N_CORES):
        out[i] = rr.results[i]["y"].reshape(C, N)
    return out
